# revision 12
# baseline (speedup 1.0000x reference)
"""nn_ConvSOM_dense1 Trainium2 kernel: 3x GCNConv + SOM scatter + dense head.

Self-contained: host prep (edge sort/pad, degree norm), Bass/Tile SPMD kernel
on 8 NeuronCores, host gather of per-core G partials + tiny final linear.

Structure per conv:
  h-phase: h' = dinv * (x @ W) in bf16, rows DMA'd to lo/hi shard tables.
  Two AllGathers (lo half after 25 tiles, hi half after all 49) so the
  collective overlaps the h-phase tail and the h0 aggregation pass.
  Aggregation in two passes: pass A consumes lo-table gathers into an SBUF
  fp32 accumulator; pass B consumes hi-table gathers, adds the partial, and
  applies bias+leaky-relu. Degree norm is folded as: dinv[src] pre-scaled
  into the table rows, dinv[dst] folded into the one-hot scatter matrices
  (tensor_scalar is_equal*mult with two per-partition scalar columns).
SOM phase-1 (distances, winner) is interleaved per-tile into conv3 pass B;
phase-2 (neighborhood weights) runs batched over all 49 tiles at the end.
Per-graph G partials summed on host + final linear + sigmoid.
"""
import numpy as np
import ml_dtypes

N = 50000
E = 800000
C = 128
P0, P1 = 16, 16
NUM_GRAPHS = 64
SIGMA = 2.0
NEG_SLOPE = 0.01
NC_ = 8
NPC = 6272            # nodes per core = 49*128
NSTAR = NC_ * NPC     # 50176
NT = NPC // 128       # 49 node tiles / core
LO = 3200             # lo rows per shard (25 tiles)
HI_ = NPC - LO        # 3072 (24 tiles)
NQ = 3                # SWDGE queues
GRP = 7               # node tiles per grouped gather (49 = 7*7)
P = 128

_CACHE = {}
TRACE = False
LAST_EXEC_NS = None
LAST_RES = None


def _host_prep(x, edge_index, batch):
    src = np.asarray(edge_index[0], dtype=np.int64)
    dst = np.asarray(edge_index[1], dtype=np.int64)
    loops = np.arange(N, dtype=np.int64)
    s = np.concatenate([src, loops])
    d = np.concatenate([dst, loops])
    deg = np.bincount(d, minlength=N).astype(np.float32)
    dinv = np.where(deg > 0, deg ** -0.5, 0.0).astype(np.float32)

    core = d // NPC
    tloc = (d % NPC) // 128
    lrow = s % NPC
    score = s // NPC
    half = (lrow >= LO).astype(np.int64)
    idxval = np.where(half == 1, score * HI_ + (lrow - LO), score * LO + lrow)
    key = core * (NT * 2) + tloc * 2 + half
    counts = np.bincount(key, minlength=NC_ * NT * 2).reshape(NC_, NT, 2)
    T = np.ceil(counts.max(axis=0) / 128).astype(np.int64)  # [NT,2]
    T[:, 0] = np.maximum(T[:, 0], 1)
    slot_sz = T * 128
    # slot order: all h=0 segments (grouped by GRP), then all h=1 segments
    seg_order = [(t, h) for h in range(2) for g in range(NT // GRP)
                 for t in range(g * GRP, (g + 1) * GRP)]
    seg_off = np.zeros((NT, 2), np.int64)
    acc = 0
    for (t, h) in seg_order:
        seg_off[t, h] = acc
        acc += int(slot_sz[t, h])
    nslots = acc
    n_et = nslots // 128

    order = np.lexsort((s, key))
    sk, ss, sd = key[order], s[order], d[order]
    grp_start = np.zeros(NC_ * NT * 2, np.int64)
    cnt_flat = counts.reshape(-1)
    grp_start[1:] = np.cumsum(cnt_flat)[:-1]
    rank = np.arange(len(sk)) - grp_start[sk]
    seg_id = sk % (NT * 2)
    slot = seg_off.reshape(-1)[seg_id] + rank
    score_core = sk // (NT * 2)
    sidx = idxval[order]

    idx_all = np.zeros((NC_, nslots), np.int16)
    dl_all = np.full((NC_, nslots), -1.0, ml_dtypes.bfloat16)
    idx_all[score_core, slot] = sidx.astype(np.int16)
    dl_all[score_core, slot] = (sd % 128).astype(np.float32)

    dvpad = np.zeros(NSTAR, np.float32)
    dvpad[:N] = dinv
    dv16 = dvpad.reshape(NC_, NT, 128).transpose(0, 2, 1).copy()

    def dvpad_f():
        return dvpad

    idx16 = np.zeros((NC_, 16, nslots // 16), np.int16)
    for t in range(NT):
        for h in range(2):
            o, sz = int(seg_off[t, h]), int(slot_sz[t, h])
            if sz == 0:
                continue
            seg = idx_all[:, o:o + sz]
            idx16[:, :, o // 16:(o + sz) // 16] = \
                seg.reshape(NC_, sz // 16, 16).transpose(0, 2, 1)
    dl16 = dl_all.reshape(NC_, n_et, 128).transpose(0, 2, 1).copy()
    ddr16 = np.broadcast_to(dvpad_f(), (128, NSTAR)).reshape(
        128, NC_, NPC).transpose(1, 0, 2).astype(ml_dtypes.bfloat16).copy()

    xpad = np.zeros((NSTAR, C), np.float32)
    xpad[:N] = np.asarray(x, np.float32)
    xT = xpad.reshape(NC_, NPC, C).transpose(0, 2, 1).astype(ml_dtypes.bfloat16).copy()

    bpad = np.full(NSTAR, -1.0, np.float32)
    bpad[:N] = np.asarray(batch, np.float32)
    batch16 = bpad.reshape(NC_, NT, 128).transpose(0, 2, 1).copy()


    return dict(T=T, n_et=n_et, idx16=idx16, dl16=dl16, ddr16=ddr16,
                xT=xT, batch16=batch16, dv16=dv16)


def _build(T, n_et):
    import concourse.bass as bass
    import concourse.bacc as bacc
    import concourse.tile as tile
    import concourse.mybir as mybir
    from concourse.library_config import mlp
    import dataclasses as _dc
    dt = mybir.dt
    AF = mybir.ActivationFunctionType
    OP = mybir.AluOpType
    INV2S2 = 1.0 / (2.0 * SIGMA * SIGMA)
    Stot = n_et * 8

    nc = bacc.Bacc("TRN2", target_bir_lowering=False, debug=False,
                   num_devices=NC_, num_swdge_queues=NQ)
    xT_d = nc.dram_tensor("xT", [P, NPC], dt.bfloat16, kind="ExternalInput")
    idx_d = nc.dram_tensor("idx16", [16, Stot], dt.int16, kind="ExternalInput")
    dl_d = nc.dram_tensor("dl16", [P, n_et], dt.bfloat16, kind="ExternalInput")
    ddr_d = nc.dram_tensor("ddr16", [P, NPC], dt.bfloat16, kind="ExternalInput")
    bt_d = nc.dram_tensor("batch16", [P, NT], dt.float32, kind="ExternalInput")
    dv_d = nc.dram_tensor("dv16", [P, NT], dt.float32, kind="ExternalInput")
    W_d = [nc.dram_tensor(f"W{k}", [C, C], dt.bfloat16, kind="ExternalInput")
           for k in (1, 2, 3)]
    b_d = [nc.dram_tensor(f"b{k}", [C, 1], dt.float32, kind="ExternalInput")
           for k in (1, 2, 3)]
    sft_d = nc.dram_tensor("SfT", [3 * C, 256], dt.bfloat16, kind="ExternalInput")
    g_out = nc.dram_tensor("g_out", [64, 256], dt.float32, kind="ExternalOutput")

    with tile.TileContext(nc) as tc:
        with (
            tc.tile_pool(name="cst", bufs=1) as cst,
            tc.tile_pool(name="xk", bufs=1) as xkp,
            tc.tile_pool(name="sb", bufs=4) as sb,
            tc.tile_pool(name="gbp", bufs=3) as gbp,
            tc.tile_pool(name="ps", bufs=3, space="PSUM") as ps,
            tc.tile_pool(name="pshs", bufs=2, space="PSUM") as pshs,
            tc.tile_pool(name="psg", bufs=1, space="PSUM") as psg,
            tc.tile_pool(name="dram", bufs=1, space="DRAM") as dram,
        ):
            nc.gpsimd.load_library(mlp)
            # ---- constant loads ----
            idx_sb = cst.tile([128, Stot], dt.int16)
            nc.vector.memset(idx_sb[:], 0)
            for q in range(NQ):
                nc.sync.dma_start(idx_sb[q * 32:q * 32 + 16, :], idx_d[:])
                nc.sync.dma_start(idx_sb[q * 32 + 16:q * 32 + 32, :], idx_d[:])
            dl_sb = cst.tile([P, n_et], dt.bfloat16)
            ddr_sb = cst.tile([P, NPC], dt.bfloat16)
            btl_sb = cst.tile([P, NT], dt.float32)
            dv_sb = cst.tile([P, NT], dt.float32)
            nc.sync.dma_start(dl_sb[:], dl_d[:])
            nc.sync.dma_start(ddr_sb[:], ddr_d[:])
            nc.sync.dma_start(btl_sb[:], bt_d[:])
            nc.sync.dma_start(dv_sb[:], dv_d[:])
            xin = cst.tile([P, NPC], dt.bfloat16)
            nc.sync.dma_start(xin[:], xT_d[:])
            W_sb, b_sb, sft_sb = [], [], []
            for k in range(3):
                w = cst.tile([C, C], dt.bfloat16, tag=f"W{k}")
                nc.sync.dma_start(w[:], W_d[k][:])
                W_sb.append(w)
                b = cst.tile([C, 1], dt.float32, tag=f"b{k}")
                nc.sync.dma_start(b[:], b_d[k][:])
                b_sb.append(b)
                sft = cst.tile([C, 256], dt.bfloat16, tag=f"sft{k}")
                nc.sync.dma_start(sft[:], sft_d[k * C:(k + 1) * C, :])
                sft_sb.append(sft)
            iota_b = cst.tile([P, P], dt.bfloat16)
            iota_i = cst.tile([P, P], dt.int32)
            nc.gpsimd.iota(iota_i[:], pattern=[[1, P]], base=0, channel_multiplier=0)
            nc.vector.tensor_copy(iota_b[:], iota_i[:])
            iota64 = cst.tile([P, 64], dt.float32)
            iota64_i = cst.tile([P, 64], dt.int32)
            nc.gpsimd.iota(iota64_i[:], pattern=[[1, 64]], base=0, channel_multiplier=0)
            nc.vector.tensor_copy(iota64[:], iota64_i[:])
            iota16 = cst.tile([P, 16], dt.float32)
            iota16_i = cst.tile([P, 16], dt.int32)
            nc.gpsimd.iota(iota16_i[:], pattern=[[1, 16]], base=0, channel_multiplier=0)
            nc.vector.tensor_copy(iota16[:], iota16_i[:])
            ones_col = cst.tile([P, 1], dt.float32)
            nc.vector.memset(ones_col[:], 1.0)
            ones_row = cst.tile([1, P], dt.bfloat16)
            nc.vector.memset(ones_row[:], 1.0)

            # srow = -0.5*||S_u||^2  [1,256]
            srow_ps = psg.tile([1, 256], dt.float32, space="PSUM", tag="srow")
            for k in range(3):
                sq0 = sb.tile([C, 256], dt.float32, tag="ssq")
                nc.scalar.activation(sq0[:], sft_sb[k][:], AF.Square)
                nc.tensor.matmul(srow_ps[:], lhsT=ones_col[:], rhs=sq0[:],
                                 start=(k == 0), stop=(k == 2))
            srow = cst.tile([1, 256], dt.bfloat16)
            nc.scalar.activation(srow[:], srow_ps[:], AF.Identity, scale=-0.5)

            x_cur = [xkp.tile([P, NPC], dt.bfloat16, tag=f"x{k}", name=f"x{k}")
                     for k in range(3)]
            xacc = xkp.tile([P, NPC], dt.float32, tag="xacc", name="xacc")
            # SOM per-node scalars, one column per node tile
            MX = cst.tile([P, NT], dt.float32)
            MI_ = cst.tile([P, NT], dt.uint32)
            HS = cst.tile([P, NT], dt.float32)

            hn_lo = [dram.tile([LO, C], dt.bfloat16, tag=f"hlo{k}",
                               name=f"hlo{k}") for k in range(3)]
            hn_hi = [dram.tile([HI_, C], dt.bfloat16, tag=f"hhi{k}",
                               name=f"hhi{k}") for k in range(3)]
            flo = [dram.tile([NC_ * LO, C], dt.bfloat16, tag=f"flo{k}",
                             name=f"flo{k}", addr_space="Shared")
                   for k in range(3)]
            fhi = [dram.tile([NC_ * HI_, C], dt.bfloat16, tag=f"fhi{k}",
                             name=f"fhi{k}", addr_space="Shared")
                   for k in range(3)]

            def onehot(J):
                oh = sb.tile([P, P], dt.bfloat16, tag="oh")
                nc.vector.tensor_tensor(
                    out=oh[:],
                    in0=dl_sb[:, J:J + 1].to_broadcast([P, P]),
                    in1=iota_b[:], op=OP.is_equal)
                return oh

            def emit_h_tile(kk, t):
                sl = slice(t * 128, (t + 1) * 128)
                lhs_x = xin[:, sl] if kk == 0 else x_cur[kk - 1][:, sl]
                h_ps = ps.tile([P, 256], dt.float32, space="PSUM", tag="w")
                nc.tensor.matmul(h_ps[:, :C], lhsT=lhs_x,
                                 rhs=W_sb[kk][:], start=True, stop=True)
                h_bf = sb.tile([P, C], dt.bfloat16, tag="hbf")
                nc.scalar.activation(h_bf[:], h_ps[:, :C], AF.Identity,
                                     scale=dv_sb[:, t:t + 1])
                if t < 25:
                    nc.sync.dma_start(hn_lo[kk][t * 128:(t + 1) * 128, :],
                                      h_bf[:])
                else:
                    tt0 = t - 25
                    nc.sync.dma_start(hn_hi[kk][tt0 * 128:(tt0 + 1) * 128, :],
                                      h_bf[:])

            def emit_ag_lo(kk):
                nc.gpsimd.collective_compute(
                    "AllGather", mybir.AluOpType.bypass,
                    replica_groups=[list(range(NC_))],
                    ins=[hn_lo[kk].opt()], outs=[flo[kk].opt()])

            def emit_ag_hi(kk):
                nc.gpsimd.collective_compute(
                    "AllGather", mybir.AluOpType.bypass,
                    replica_groups=[list(range(NC_))],
                    ins=[hn_hi[kk].opt()], outs=[fhi[kk].opt()])

            for t0_ in range(NT):
                emit_h_tile(0, t0_)
                if t0_ == 24:
                    emit_ag_lo(0)
            emit_ag_hi(0)

            gq = 0
            for k in range(3):
                # ---- pass A: lo-half edges -> xacc ----
                for (sa, sb_) in _SUBS:
                    tiles = list(range(sa, sb_))
                    Tg = int(sum(T[t, 0] for t in tiles))
                    o = int(_SEGOFF[tiles[0]][0])
                    gbA = gbp.tile([P, _TGMAX, P], dt.bfloat16, tag="gA",
                                   name="gA")
                    nc.gpsimd.dma_gather(
                        out_ap=gbA[:, :Tg, :], in_ap=flo[k][:],
                        idxs_ap=idx_sb[:, o // 16:(o + Tg * 128) // 16],
                        num_idxs=Tg * 128, num_idxs_reg=Tg * 128,
                        elem_size=P, single_packet=False, queue_num=gq % NQ)
                    gq += 1
                    for t in tiles:
                        sl = slice(t * 128, (t + 1) * 128)
                        base = (int(_SEGOFF[t][0]) - o) // 128
                        J0 = int(_SEGOFF[t][0]) // 128
                        nT = int(T[t, 0])
                        psA = ps.tile([P, 256], dt.float32, space="PSUM", tag="w")
                        for tt in range(nT):
                            oh = onehot(J0 + tt)
                            nc.tensor.matmul(psA[:, :P], lhsT=gbA[:, base + tt, :],
                                             rhs=oh[:], start=(tt == 0),
                                             stop=(tt == nT - 1))
                        nc.scalar.activation(xacc[:, sl], psA[:, :P], AF.Identity)

                # ---- pass B: hi-half edges + bias + lrelu ----
                for (sa, sb_) in _SUBS:
                    tiles = list(range(sa, sb_))
                    Tg = int(sum(T[t, 1] for t in tiles))
                    o = int(_SEGOFF[tiles[0]][1])
                    if Tg > 0:
                        gbB = gbp.tile([P, _TGMAX, P], dt.bfloat16, tag="gB",
                                       name="gB")
                        nc.gpsimd.dma_gather(
                            out_ap=gbB[:, :Tg, :], in_ap=fhi[k][:],
                            idxs_ap=idx_sb[:, o // 16:(o + Tg * 128) // 16],
                            num_idxs=Tg * 128, num_idxs_reg=Tg * 128,
                            elem_size=P, single_packet=False, queue_num=gq % NQ)
                        gq += 1
                    for t in tiles:
                        sl = slice(t * 128, (t + 1) * 128)
                        nT = int(T[t, 1])
                        if nT > 0:
                            base = (int(_SEGOFF[t][1]) - o) // 128
                            J0 = int(_SEGOFF[t][1]) // 128
                            psB = ps.tile([P, 256], dt.float32, space="PSUM",
                                          tag="w")
                            for tt in range(nT):
                                oh = onehot(J0 + tt)
                                nc.tensor.matmul(psB[:, :P],
                                                 lhsT=gbB[:, base + tt, :],
                                                 rhs=oh[:], start=(tt == 0),
                                                 stop=(tt == nT - 1))
                            nc.vector.tensor_add(xacc[:, sl], psB[:, :P],
                                                 xacc[:, sl])
                        nc.vector.tensor_tensor(out=xacc[:, sl],
                                                in0=xacc[:, sl],
                                                in1=ddr_sb[:, sl], op=OP.mult)
                        nc.scalar.activation(x_cur[k][:, sl], xacc[:, sl],
                                             AF.Lrelu, bias=b_sb[k][:, :1],
                                             alpha=NEG_SLOPE)
                        if k < 2:
                            emit_h_tile(k + 1, t)
                            if t == 24:
                                emit_ag_lo(k + 1)
                            if t == NT - 1:
                                emit_ag_hi(k + 1)
                        if k == 2:
                            # SOM phase-1 for this tile (overlaps gathers)
                            D_ps = ps.tile([P, 256], dt.float32, space="PSUM",
                                           tag="w")
                            hs_ps = pshs.tile([P, 1], dt.float32, space="PSUM",
                                              tag="hs")
                            for kk in range(3):
                                nc.tensor.matmul(D_ps[:], lhsT=x_cur[kk][:, sl],
                                                 rhs=sft_sb[kk][:],
                                                 start=(kk == 0), stop=False,
                                                 skip_group_check=True)
                                sq = sb.tile([P, P], dt.float32, tag="xsq")
                                nc.vector.tensor_tensor(
                                    out=sq[:], in0=x_cur[kk][:, sl],
                                    in1=x_cur[kk][:, sl], op=OP.mult)
                                nc.tensor.matmul(hs_ps[:], lhsT=sq[:],
                                                 rhs=ones_col[:],
                                                 start=(kk == 0), stop=(kk == 2),
                                                 skip_group_check=True)
                            nc.tensor.matmul(D_ps[:], lhsT=ones_row[:],
                                             rhs=srow[:], start=False, stop=True,
                                             skip_group_check=True)
                            mx = sb.tile([P, 8], dt.float32, tag="mx")
                            mi = sb.tile([P, 8], dt.uint32, tag="mi")
                            nc.vector.max_with_indices(mx[:], mi[:], D_ps[:])
                            nc.vector.tensor_copy(MX[:, t:t + 1], mx[:, :1])
                            nc.vector.tensor_copy(MI_[:, t:t + 1], mi[:, :1])
                            nc.vector.tensor_copy(HS[:, t:t + 1], hs_ps[:])

            # ---- SOM phase 2 (batched) ----
            WJu = cst.tile([P, NT], dt.uint32, tag="wju")
            WIu = cst.tile([P, NT], dt.uint32, tag="wiu")
            nc.vector.tensor_scalar(out=WJu[:], in0=MI_[:], scalar1=15,
                                    scalar2=None, op0=OP.bitwise_and)
            nc.vector.tensor_scalar(out=WIu[:], in0=MI_[:], scalar1=4,
                                    scalar2=None, op0=OP.logical_shift_right)
            WJF = cst.tile([P, NT], dt.float32, tag="wjf")
            WIF = cst.tile([P, NT], dt.float32, tag="wif")
            nc.vector.tensor_copy(WJF[:], WJu[:])
            nc.vector.tensor_copy(WIF[:], WIu[:])
            D2 = cst.tile([P, NT], dt.float32, tag="d2")
            nc.vector.tensor_scalar(out=D2[:], in0=MX[:], scalar1=-2.0,
                                    scalar2=None, op0=OP.mult)
            nc.vector.tensor_add(D2[:], D2[:], HS[:])
            nc.vector.tensor_scalar_max(D2[:], D2[:], 0.0)
            MIND = cst.tile([P, NT], dt.float32, tag="mind")
            nc.scalar.activation(MIND[:], D2[:], AF.Sqrt)
            HSV = cst.tile([P, NT], dt.float32, tag="hsv")
            nc.scalar.activation(HSV[:], MIND[:], AF.Exp, scale=-1.0)

            AX = cst.tile([P, NT * 16], dt.float32, tag="ax")
            AY = cst.tile([P, NT * 16], dt.float32, tag="ay")
            i16 = iota16[:]
            i16b = _dc.replace(i16, ap=[i16.ap[0], [0, NT], i16.ap[1]])
            for (W_f, out_t) in ((WIF, AX), (WJF, AY)):
                wap = W_f[:]
                wb = _dc.replace(wap, ap=[wap.ap[0], wap.ap[1], [0, 16]])
                nc.vector.tensor_tensor(out=out_t[:], in0=i16b, in1=wb,
                                        op=OP.subtract)
                nc.vector.tensor_tensor(out=out_t[:], in0=out_t[:],
                                        in1=out_t[:], op=OP.mult)
                nc.scalar.activation(out_t[:], out_t[:], AF.Exp, scale=-INV2S2)
            hap = HSV[:]
            hb = _dc.replace(hap, ap=[hap.ap[0], hap.ap[1], [0, 16]])
            nc.vector.tensor_tensor(out=AX[:], in0=AX[:], in1=hb, op=OP.mult)

            G_ps = psg.tile([64, 256], dt.float32, space="PSUM", tag="G")
            for t in range(NT):
                axs = AX[:, t * 16:(t + 1) * 16]
                ays = AY[:, t * 16:(t + 1) * 16]
                axb = _dc.replace(axs, ap=[axs.ap[0], axs.ap[1], [0, 16]])
                ayb = _dc.replace(ays, ap=[ays.ap[0], [0, 16], ays.ap[1]])
                contrib = sb.tile([P, 256], dt.bfloat16, tag="contrib")
                nc.vector.tensor_tensor(out=contrib[:], in0=axb, in1=ayb,
                                        op=OP.mult)
                bt = sb.tile([P, 64], dt.bfloat16, tag="bt")
                nc.vector.tensor_tensor(
                    out=bt[:],
                    in0=btl_sb[:, t:t + 1].to_broadcast([P, 64]),
                    in1=iota64[:], op=OP.is_equal)
                nc.tensor.matmul(G_ps[:], lhsT=bt[:], rhs=contrib[:],
                                 start=(t == 0), stop=(t == NT - 1),
                                 skip_group_check=True)
            G_sb = cst.tile([64, 256], dt.float32)
            nc.scalar.activation(G_sb[:], G_ps[:], AF.Identity)
            nc.sync.dma_start(g_out[:], G_sb[:])
    nc.compile()
    return nc


_SEGOFF = None
_TGMAX = None
_SUBS = []
for _g in range(NT // GRP):
    _SUBS += [(_g * GRP, _g * GRP + 4), (_g * GRP + 4, (_g + 1) * GRP)]


def kernel(**inputs):
    global _SEGOFF, _TGMAX
    from concourse.bass_utils import run_bass_kernel_spmd

    x = np.asarray(inputs["x"], np.float32)
    prep = _host_prep(x, np.asarray(inputs["edge_index"]),
                      np.asarray(inputs["batch"]))
    T, n_et = prep["T"], prep["n_et"]
    seg_order = [(t, h) for h in range(2) for g in range(NT // GRP)
                 for t in range(g * GRP, (g + 1) * GRP)]
    seg_off = np.zeros((NT, 2), np.int64)
    acc = 0
    for (t, h) in seg_order:
        seg_off[t, h] = acc
        acc += int(T[t, h]) * 128
    _SEGOFF = seg_off
    _TGMAX = int(max(T[a:b, h].sum()
                     for (a, b) in _SUBS for h in range(2)))

    ck = (n_et, tuple(T.reshape(-1).tolist()))
    if ck not in _CACHE:
        _CACHE[ck] = _build(T, n_et)
    nc = _CACHE[ck]

    SfT = np.asarray(inputs["S"], np.float32).reshape(256, 384).T
    SfT = SfT.astype(ml_dtypes.bfloat16).copy()
    in_maps = []
    for c in range(NC_):
        m = dict(
            xT=prep["xT"][c], idx16=prep["idx16"][c], dl16=prep["dl16"][c],
            ddr16=prep["ddr16"][c], batch16=prep["batch16"][c],
            dv16=prep["dv16"][c],
            SfT=SfT,
            W1=np.asarray(inputs["W1"], np.float32).astype(ml_dtypes.bfloat16),
            W2=np.asarray(inputs["W2"], np.float32).astype(ml_dtypes.bfloat16),
            W3=np.asarray(inputs["W3"], np.float32).astype(ml_dtypes.bfloat16),
            b1=np.asarray(inputs["b1"], np.float32).reshape(C, 1),
            b2=np.asarray(inputs["b2"], np.float32).reshape(C, 1),
            b3=np.asarray(inputs["b3"], np.float32).reshape(C, 1),
        )
        in_maps.append(m)
    global LAST_EXEC_NS, LAST_RES
    kw = {}
    if TRACE:
        kw = dict(trace=True)
    res = run_bass_kernel_spmd(nc, in_maps, core_ids=list(range(NC_)), **kw)
    LAST_RES = res
    LAST_EXEC_NS = res.exec_time_ns
    G = np.zeros((64, 256), np.float64)
    for c in range(NC_):
        G += res.results[c]["g_out"].astype(np.float64)
    lin_W = np.asarray(inputs["lin_W"], np.float32)
    lin_b = np.asarray(inputs["lin_b"], np.float32)
    z = G.astype(np.float32) @ lin_W.T + lin_b
    return (1.0 / (1.0 + np.exp(-z))).astype(np.float32)
